# revision 11
# baseline (speedup 1.0000x reference)
"""Token-sharded (data-parallel) Trainium2 Bass kernel for
nn_Linear_80874234183916.

y = x @ w_eff.T + bias, w_eff = weight masked to the top-half magnitudes
(threshold = median |w|), x [8192, 4096], w [4096, 4096].

Sharding: 8-way data-parallel on the token dim. Each core owns 1024 tokens:
its x slice (8 MiB bf16) is DMA'd once and stays RESIDENT in SBUF; the full
weight (32 MiB bf16) streams through a deep tile pool at only ~73 GB/s
sustained — 2x less HBM pressure than the tensor-parallel layout (which must
stream x at 146 GB/s for the whole exec), so the PE stays fed even under
HBM contention from co-tenants.

Per-core schedule (w-stationary, nb-PAIRS for 4-way PSUM chain ILP — with
only 2 parallel chains the PE measurably stalls ~10%):
  for nb-pair in 16 (2x 128-out-feature blocks, one 2 MiB w DMA each,
                     double-buffered through a 4-tile pool):
    for ko in 32: LDW wt[nb][:,ko] x2; 4 MMs of [128k x 128m x 512t]
      accumulating into 4 PSUM banks (chains over ko, start/stop flags)
    ACT: bias-add 4 banks -> one [128, 2, 1024] bf16 tile -> ONE 512 KiB
      out DMA per pair
Totals/core: 2048 MMs (PE roofline ~437us @2.4GHz, measured at roofline in
steady state), 16 w DMAs + 8 x DMAs + 16 out DMAs + bias = 41 descriptors
(vs ~209 for the TP layout — less per-exec runtime patching work).

Numerics identical to the tensor-parallel baseline: host-side fp32
threshold/mask, bf16 operands, fp32 PSUM accumulation, ACT bias-add.
Measured rel err 2.8e-3 (gate 2e-2).
"""

import numpy as np
import ml_dtypes

import concourse.mybir as mybir
import concourse.tile as tile
from concourse import bacc
from concourse.bass_utils import run_bass_kernel_spmd

N_TOK = 8192
IN_F = 4096
OUT_F = 4096
N_CORES = 8
T_S = N_TOK // N_CORES   # 1024 tokens per core
P = 128
KO = IN_F // P           # 32 k-chunks
NB = OUT_F // P          # 32 out-feature blocks (full out dim per core)
TCH = 512                # tokens per matmul (moving dim)
CT = T_S // TCH          # 2 token slabs per core
XG = 4                   # ko per resident-x tile (8 tiles of 1 MiB)
MAX_ITER = IN_F * OUT_F // 2
TT = 64                  # tau scaling for test.py (64 token-tile units/pass)

dt = mybir.dt
BF16 = ml_dtypes.bfloat16


def _build(reps: int = 1, w_bufs: int = 8, nb_par: int = 2, xg: int = XG,
           x_on_act: bool = False):
    nc = bacc.Bacc("TRN2", target_bir_lowering=False, debug=False)

    # Host layouts (see _prep_inputs):
    #   xq[ki, ko, t] = x_slice[t, ko*128 + ki]          (bf16, 8 MiB)
    #   wq[nb, ki, ko, m] = w_eff[nb*128 + m, ko*128+ki] (bf16, 32 MiB)
    #   bt[p, nb] = bias[nb*128 + p]                     (f32)
    #   yt[nb, p, t] = y[t, nb*128 + p]                  (bf16 out, 8 MiB)
    xq = nc.dram_tensor("xq", [P, KO, T_S], dt.bfloat16, kind="ExternalInput").ap()
    wq = nc.dram_tensor("wq", [NB, P, KO, P], dt.bfloat16, kind="ExternalInput").ap()
    bt = nc.dram_tensor("bt", [P, NB], dt.float32, kind="ExternalInput").ap()
    yt = nc.dram_tensor("yt", [NB, P, T_S], dt.bfloat16, kind="ExternalOutput").ap()

    n_xg = KO // xg

    with tile.TileContext(nc) as tc:
        with (
            tc.tile_pool(name="xr", bufs=n_xg) as xrpool,
            tc.tile_pool(name="wpool", bufs=w_bufs // 2) as wpool,
            tc.tile_pool(name="opool", bufs=4) as opool,
            tc.tile_pool(name="cpool", bufs=1) as cpool,
            tc.tile_pool(name="pspool", bufs=8, space="PSUM") as ps,
        ):
            wts = {}

            def load_w(rep, nb):
                # one DMA per nb-pair (2 MiB): fewer descriptors to patch
                # per exec, same bytes/order
                assert nb % 2 == 0
                wt = wpool.tile([P, 2, KO, P], dt.bfloat16, tag="wt",
                                name=f"wt{rep}_{nb}")
                nc.sync.dma_start(
                    wt[:], wq[nb : nb + 2].rearrange("n p k m -> p n k m")
                )
                wts[nb] = wt
                wts[nb + 1] = wt

            xrs = {}

            def load_x(rep, g):
                xt = xrpool.tile([P, xg, T_S], dt.bfloat16, tag="x",
                                 name=f"x{rep}_{g}")
                # x_on_act=True routes x via the ACT HWDGE ring to overlap
                # with the w stream on SP — measured NOT faster (A/B medians
                # 648 vs 698us), so default stays SP.
                eng = nc.scalar if x_on_act else nc.sync
                eng.dma_start(xt[:], xq[:, g * xg : (g + 1) * xg])
                xrs[g] = xt

            def xslice(ko, ct):
                return xrs[ko // xg][:, ko % xg, ct * TCH : (ct + 1) * TCH]

            bias_sb = cpool.tile([P, NB], dt.float32, tag="bias")

            for _rep in range(reps):
                # Prologue per rep: first w tiles and the resident x in PE
                # consumption order (x group g is first needed by ko=g*XG).
                load_w(_rep, 0)
                load_x(_rep, 0)
                load_x(_rep, 1)
                for g in range(2, n_xg):
                    load_x(_rep, g)
                if _rep == 0:
                    nc.sync.dma_start(bias_sb[:], bt)
                for nb in range(2, w_bufs, 2):
                    load_w(_rep, nb)

                for nb0 in range(0, NB, nb_par):
                    nbs = range(nb0, nb0 + nb_par)
                    for nb in nbs:
                        if nb % 2 == 0 and nb + w_bufs < NB:
                            load_w(_rep, nb + w_bufs)
                    pss = {
                        (nb, ct): ps.tile([P, TCH], dt.float32, tag="ps",
                                          name=f"ps{_rep}_{nb}_{ct}")
                        for nb in nbs
                        for ct in range(CT)
                    }
                    for ko in range(KO):
                        for nb in nbs:
                            for ct in range(CT):
                                nc.tensor.matmul(
                                    pss[nb, ct][:],
                                    wts[nb][:, nb % 2, ko],
                                    xslice(ko, ct),
                                    start=(ko == 0),
                                    stop=(ko == KO - 1),
                                )
                    # drain the whole nb-pair into one tile -> ONE out DMA
                    # (512 KiB) on the ACT HWDGE ring, decoupled from the
                    # x/w input stream on the SP ring.
                    o = opool.tile([P, nb_par, T_S], dt.bfloat16, tag="out",
                                   name=f"o{_rep}_{nb0}")
                    for i, nb in enumerate(nbs):
                        for ct in range(CT):
                            nc.scalar.add(o[:, i, ct * TCH : (ct + 1) * TCH],
                                          pss[nb, ct][:], bias_sb[:, nb : nb + 1])
                    nc.scalar.dma_start(
                        yt[nb0 : nb0 + nb_par].rearrange("n p t -> p n t"), o[:]
                    )
                    for nb in nbs:
                        del wts[nb]

    nc.compile()
    return nc


def _prep_inputs(x, weight, bias):
    """Host-side: threshold (fp32), mask+cast to bf16, DMA-friendly tiling."""
    flat_abs = np.abs(weight.reshape(-1))
    k = flat_abs.size - MAX_ITER
    thresh = float(np.partition(flat_abs, k)[k])

    w_eff = (weight * (np.abs(weight) >= thresh)).astype(BF16)

    # wq[nb, ki, ko, m] = w_eff[nb*128+m, ko*128+ki]  (identical on all cores)
    wq = np.ascontiguousarray(
        w_eff.reshape(NB, P, KO, P).transpose(0, 3, 2, 1)
    )
    # bt[p, nb] = bias[nb*128+p]
    bt = np.ascontiguousarray(bias.reshape(NB, P).T).astype(np.float32)

    xb = x.astype(BF16)
    in_maps = []
    for c in range(N_CORES):
        # xq[ki, ko, t] = x[c*1024 + t, ko*128 + ki]
        xs = xb[c * T_S : (c + 1) * T_S]  # [1024, 4096]
        xq = np.ascontiguousarray(xs.reshape(T_S, KO, P).transpose(2, 1, 0))
        in_maps.append({"xq": xq, "wq": wq, "bt": bt})
    return thresh, in_maps


def _unshard(results):
    # yt[nb, p, t] per core -> y[c*1024 + t, nb*128 + p]
    return np.ascontiguousarray(
        np.concatenate(
            [r["yt"].transpose(2, 0, 1).reshape(T_S, OUT_F).astype(np.float32)
             for r in results],
            axis=0,
        )
    )


def _run(x, weight, bias, **run_kwargs):
    x = np.asarray(x, dtype=np.float32)
    weight = np.asarray(weight, dtype=np.float32)
    bias = np.asarray(bias, dtype=np.float32)
    assert x.shape == (N_TOK, IN_F) and weight.shape == (OUT_F, IN_F)

    _, in_maps = _prep_inputs(x, weight, bias)
    nc = _build()
    res = run_bass_kernel_spmd(
        nc, in_maps, core_ids=list(range(N_CORES)), **run_kwargs
    )
    return _unshard(res.results), res


def kernel(x, weight, bias):
    y, _ = _run(x, weight, bias)
    return y


# revision 13
# speedup vs baseline: 1.0080x; 1.0080x over previous
"""Token-sharded (data-parallel) Trainium2 Bass kernel for
nn_Linear_80874234183916.

y = x @ w_eff.T + bias, w_eff = weight masked to the top-half magnitudes
(threshold = median |w|), x [8192, 4096], w [4096, 4096].

Sharding: 8-way data-parallel on the token dim. Each core owns 1024 tokens:
its x slice (8 MiB bf16) is DMA'd once and stays RESIDENT in SBUF; the full
weight (32 MiB bf16) streams through a deep tile pool at only ~73 GB/s
sustained — 2x less HBM pressure than the tensor-parallel layout (which must
stream x at 146 GB/s for the whole exec), so the PE stays fed even under
HBM contention from co-tenants.

Per-core schedule (w-stationary, nb-PAIRS for 4-way PSUM chain ILP — with
only 2 parallel chains the PE measurably stalls ~10%):
  for nb-pair in 16 (2x 128-out-feature blocks, one 2 MiB w DMA each,
                     double-buffered through a 4-tile pool):
    for ko in 32: LDW wt[nb][:,ko] x2; 4 MMs of [128k x 128m x 512t]
      accumulating into 4 PSUM banks (chains over ko, start/stop flags)
    ACT: bias-add 4 banks -> one [128, 2, 1024] bf16 tile -> ONE 512 KiB
      out DMA per pair
Totals/core: 2048 MMs (PE roofline ~437us @2.4GHz, measured at roofline in
steady state), 16 w DMAs + 8 x DMAs + 16 out DMAs + bias = 41 descriptors
(vs ~209 for the TP layout — less per-exec runtime patching work).

Numerics identical to the tensor-parallel baseline: host-side fp32
threshold/mask, bf16 operands, fp32 PSUM accumulation, ACT bias-add.
Measured rel err 2.8e-3 (gate 2e-2).
"""

import numpy as np
import ml_dtypes

import concourse.mybir as mybir
import concourse.tile as tile
from concourse import bacc
from concourse.bass_utils import run_bass_kernel_spmd

N_TOK = 8192
IN_F = 4096
OUT_F = 4096
N_CORES = 8
T_S = N_TOK // N_CORES   # 1024 tokens per core
P = 128
KO = IN_F // P           # 32 k-chunks
NB = OUT_F // P          # 32 out-feature blocks (full out dim per core)
TCH = 512                # tokens per matmul (moving dim)
CT = T_S // TCH          # 2 token slabs per core
XG = 4                   # ko per resident-x tile (8 tiles of 1 MiB)
MAX_ITER = IN_F * OUT_F // 2
TT = 64                  # tau scaling for test.py (64 token-tile units/pass)

dt = mybir.dt
BF16 = ml_dtypes.bfloat16


def _build(reps: int = 1, w_bufs: int = 8, nb_par: int = 2, xg: int = XG,
           x_on_act: bool = False, chunk_first: bool = True):
    nc = bacc.Bacc("TRN2", target_bir_lowering=False, debug=False)

    # Host layouts (see _prep_inputs):
    #   xq[ki, ko, t] = x_slice[t, ko*128 + ki]          (bf16, 8 MiB)
    #   wq[nb, ki, ko, m] = w_eff[nb*128 + m, ko*128+ki] (bf16, 32 MiB)
    #   bt[p, nb] = bias[nb*128 + p]                     (f32)
    #   yt[nb, p, t] = y[t, nb*128 + p]                  (bf16 out, 8 MiB)
    xq = nc.dram_tensor("xq", [P, KO, T_S], dt.bfloat16, kind="ExternalInput").ap()
    wq = nc.dram_tensor("wq", [NB, P, KO, P], dt.bfloat16, kind="ExternalInput").ap()
    bt = nc.dram_tensor("bt", [P, NB], dt.float32, kind="ExternalInput").ap()
    yt = nc.dram_tensor("yt", [NB, P, T_S], dt.bfloat16, kind="ExternalOutput").ap()

    n_xg = KO // xg

    with tile.TileContext(nc) as tc:
        with (
            tc.tile_pool(name="xr", bufs=n_xg) as xrpool,
            tc.tile_pool(name="wpool", bufs=w_bufs // 2) as wpool,
            tc.tile_pool(name="opool", bufs=4) as opool,
            tc.tile_pool(name="cpool", bufs=1) as cpool,
            tc.tile_pool(name="pspool", bufs=8, space="PSUM") as ps,
        ):
            wts = {}

            def load_w(rep, nb):
                # one DMA per nb-pair (2 MiB): fewer descriptors to patch
                # per exec, same bytes/order
                assert nb % 2 == 0
                wt = wpool.tile([P, 2, KO, P], dt.bfloat16, tag="wt",
                                name=f"wt{rep}_{nb}")
                if chunk_first and rep == 0 and nb == 0:
                    # pair 0 in 4 ko-range chunks so the first chains gate on
                    # 512 KiB instead of the full 2 MiB (earlier PE start;
                    # correct either way — if Tile's slice intersection is
                    # coarse this degrades to the unchunked behavior)
                    kq = KO // 4
                    src = wq[nb : nb + 2].rearrange("n p k m -> p n k m")
                    for c in range(4):
                        nc.sync.dma_start(
                            wt[:, :, c * kq : (c + 1) * kq],
                            src[:, :, c * kq : (c + 1) * kq],
                        )
                else:
                    nc.sync.dma_start(
                        wt[:], wq[nb : nb + 2].rearrange("n p k m -> p n k m")
                    )
                wts[nb] = wt
                wts[nb + 1] = wt

            xrs = {}

            def load_x(rep, g):
                xt = xrpool.tile([P, xg, T_S], dt.bfloat16, tag="x",
                                 name=f"x{rep}_{g}")
                # x_on_act=True routes x via the ACT HWDGE ring to overlap
                # with the w stream on SP — measured NOT faster (A/B medians
                # 648 vs 698us), so default stays SP.
                eng = nc.scalar if x_on_act else nc.sync
                eng.dma_start(xt[:], xq[:, g * xg : (g + 1) * xg])
                xrs[g] = xt

            def xslice(ko, ct):
                return xrs[ko // xg][:, ko % xg, ct * TCH : (ct + 1) * TCH]

            bias_sb = cpool.tile([P, NB], dt.float32, tag="bias")

            for _rep in range(reps):
                # Prologue per rep: first w tiles and the resident x in PE
                # consumption order (x group g is first needed by ko=g*XG).
                load_w(_rep, 0)
                load_x(_rep, 0)
                load_x(_rep, 1)
                for g in range(2, n_xg):
                    load_x(_rep, g)
                if _rep == 0:
                    nc.sync.dma_start(bias_sb[:], bt)
                for nb in range(2, w_bufs, 2):
                    load_w(_rep, nb)

                for nb0 in range(0, NB, nb_par):
                    nbs = range(nb0, nb0 + nb_par)
                    for nb in nbs:
                        if nb % 2 == 0 and nb + w_bufs < NB:
                            load_w(_rep, nb + w_bufs)
                    pss = {
                        (nb, ct): ps.tile([P, TCH], dt.float32, tag="ps",
                                          name=f"ps{_rep}_{nb}_{ct}")
                        for nb in nbs
                        for ct in range(CT)
                    }
                    for ko in range(KO):
                        for nb in nbs:
                            for ct in range(CT):
                                nc.tensor.matmul(
                                    pss[nb, ct][:],
                                    wts[nb][:, nb % 2, ko],
                                    xslice(ko, ct),
                                    start=(ko == 0),
                                    stop=(ko == KO - 1),
                                )
                    # drain the whole nb-pair into one tile -> ONE out DMA
                    # (512 KiB) on the ACT HWDGE ring, decoupled from the
                    # x/w input stream on the SP ring.
                    o = opool.tile([P, nb_par, T_S], dt.bfloat16, tag="out",
                                   name=f"o{_rep}_{nb0}")
                    for i, nb in enumerate(nbs):
                        for ct in range(CT):
                            nc.scalar.add(o[:, i, ct * TCH : (ct + 1) * TCH],
                                          pss[nb, ct][:], bias_sb[:, nb : nb + 1])
                    nc.scalar.dma_start(
                        yt[nb0 : nb0 + nb_par].rearrange("n p t -> p n t"), o[:]
                    )
                    for nb in nbs:
                        del wts[nb]

    nc.compile()
    return nc


def _prep_inputs(x, weight, bias):
    """Host-side: threshold (fp32), mask+cast to bf16, DMA-friendly tiling."""
    flat_abs = np.abs(weight.reshape(-1))
    k = flat_abs.size - MAX_ITER
    thresh = float(np.partition(flat_abs, k)[k])

    w_eff = (weight * (np.abs(weight) >= thresh)).astype(BF16)

    # wq[nb, ki, ko, m] = w_eff[nb*128+m, ko*128+ki]  (identical on all cores)
    wq = np.ascontiguousarray(
        w_eff.reshape(NB, P, KO, P).transpose(0, 3, 2, 1)
    )
    # bt[p, nb] = bias[nb*128+p]
    bt = np.ascontiguousarray(bias.reshape(NB, P).T).astype(np.float32)

    xb = x.astype(BF16)
    in_maps = []
    for c in range(N_CORES):
        # xq[ki, ko, t] = x[c*1024 + t, ko*128 + ki]
        xs = xb[c * T_S : (c + 1) * T_S]  # [1024, 4096]
        xq = np.ascontiguousarray(xs.reshape(T_S, KO, P).transpose(2, 1, 0))
        in_maps.append({"xq": xq, "wq": wq, "bt": bt})
    return thresh, in_maps


def _unshard(results):
    # yt[nb, p, t] per core -> y[c*1024 + t, nb*128 + p]
    return np.ascontiguousarray(
        np.concatenate(
            [r["yt"].transpose(2, 0, 1).reshape(T_S, OUT_F).astype(np.float32)
             for r in results],
            axis=0,
        )
    )


def _run(x, weight, bias, **run_kwargs):
    x = np.asarray(x, dtype=np.float32)
    weight = np.asarray(weight, dtype=np.float32)
    bias = np.asarray(bias, dtype=np.float32)
    assert x.shape == (N_TOK, IN_F) and weight.shape == (OUT_F, IN_F)

    _, in_maps = _prep_inputs(x, weight, bias)
    nc = _build()
    res = run_bass_kernel_spmd(
        nc, in_maps, core_ids=list(range(N_CORES)), **run_kwargs
    )
    return _unshard(res.results), res


def kernel(x, weight, bias):
    y, _ = _run(x, weight, bias)
    return y


# revision 14
# speedup vs baseline: 1.0222x; 1.0141x over previous
"""Token-sharded (data-parallel) Trainium2 Bass kernel for
nn_Linear_80874234183916.

y = x @ w_eff.T + bias, w_eff = weight masked to the top-half magnitudes
(threshold = median |w|), x [8192, 4096], w [4096, 4096].

Sharding: 8-way data-parallel on the token dim. Each core owns 1024 tokens:
its x slice (8 MiB bf16) is DMA'd once and stays RESIDENT in SBUF; the full
weight (32 MiB bf16) streams through a deep tile pool at only ~73 GB/s
sustained — 2x less HBM pressure than the tensor-parallel layout (which must
stream x at 146 GB/s for the whole exec), so the PE stays fed even under
HBM contention from co-tenants.

Per-core schedule (w-stationary, nb-PAIRS for 4-way PSUM chain ILP — with
only 2 parallel chains the PE measurably stalls ~10%):
  for nb-pair in 16 (2x 128-out-feature blocks, one 2 MiB w DMA each,
                     double-buffered through a 4-tile pool):
    for ko in 32: LDW wt[nb][:,ko] x2; 4 MMs of [128k x 128m x 512t]
      accumulating into 4 PSUM banks (chains over ko, start/stop flags)
    ACT: bias-add 4 banks -> one [128, 2, 1024] bf16 tile -> ONE 512 KiB
      out DMA per pair
Totals/core: 2048 MMs (PE roofline ~437us @2.4GHz, measured at roofline in
steady state), 16 w DMAs + 8 x DMAs + 16 out DMAs + bias = 41 descriptors
(vs ~209 for the TP layout — less per-exec runtime patching work).

Numerics identical to the tensor-parallel baseline: host-side fp32
threshold/mask, bf16 operands, fp32 PSUM accumulation, ACT bias-add.
Measured rel err 2.8e-3 (gate 2e-2).
"""

import numpy as np
import ml_dtypes

import concourse.mybir as mybir
import concourse.tile as tile
from concourse import bacc
from concourse.bass_utils import run_bass_kernel_spmd

N_TOK = 8192
IN_F = 4096
OUT_F = 4096
N_CORES = 8
T_S = N_TOK // N_CORES   # 1024 tokens per core
P = 128
KO = IN_F // P           # 32 k-chunks
NB = OUT_F // P          # 32 out-feature blocks (full out dim per core)
TCH = 512                # tokens per matmul (moving dim)
CT = T_S // TCH          # 2 token slabs per core
XG = 4                   # ko per resident-x tile (8 tiles of 1 MiB)
MAX_ITER = IN_F * OUT_F // 2
TT = 64                  # tau scaling for test.py (64 token-tile units/pass)

dt = mybir.dt
BF16 = ml_dtypes.bfloat16


def _build(reps: int = 1, w_bufs: int = 8, nb_par: int = 2, xg: int = XG,
           x_on_act: bool = False, chunk_first: bool = True):
    nc = bacc.Bacc("TRN2", target_bir_lowering=False, debug=False)

    # Host layouts (see _prep_inputs):
    #   xq[ki, ko, t] = x_slice[t, ko*128 + ki]          (bf16, 8 MiB)
    #   wq[nb, ki, ko, m] = w_eff[nb*128 + m, ko*128+ki] (bf16, 32 MiB)
    #   bt[p, nb] = bias[nb*128 + p]                     (f32)
    #   yt[nb, p, t] = y[t, nb*128 + p]                  (bf16 out, 8 MiB)
    xq = nc.dram_tensor("xq", [P, KO, T_S], dt.bfloat16, kind="ExternalInput").ap()
    wq = nc.dram_tensor("wq", [NB, P, KO, P], dt.bfloat16, kind="ExternalInput").ap()
    bt = nc.dram_tensor("bt", [P, NB], dt.float32, kind="ExternalInput").ap()
    yt = nc.dram_tensor("yt", [NB, P, T_S], dt.bfloat16, kind="ExternalOutput").ap()

    n_xg = KO // xg

    with tile.TileContext(nc) as tc:
        with (
            tc.tile_pool(name="xr", bufs=n_xg) as xrpool,
            tc.tile_pool(name="wpool", bufs=w_bufs // 2) as wpool,
            tc.tile_pool(name="opool", bufs=4) as opool,
            tc.tile_pool(name="cpool", bufs=1) as cpool,
            tc.tile_pool(name="pspool", bufs=8, space="PSUM") as ps,
        ):
            wts = {}

            def load_w(rep, nb):
                # one DMA per nb-pair (2 MiB): fewer descriptors to patch
                # per exec, same bytes/order
                assert nb % 2 == 0
                wt = wpool.tile([P, 2, KO, P], dt.bfloat16, tag="wt",
                                name=f"wt{rep}_{nb}")
                if chunk_first and rep == 0 and nb == 0:
                    # pair 0 in 4 ko-range chunks so the first chains gate on
                    # 512 KiB instead of the full 2 MiB (earlier PE start;
                    # correct either way — if Tile's slice intersection is
                    # coarse this degrades to the unchunked behavior)
                    kq = KO // 4
                    src = wq[nb : nb + 2].rearrange("n p k m -> p n k m")
                    for c in range(4):
                        nc.sync.dma_start(
                            wt[:, :, c * kq : (c + 1) * kq],
                            src[:, :, c * kq : (c + 1) * kq],
                        )
                else:
                    nc.sync.dma_start(
                        wt[:], wq[nb : nb + 2].rearrange("n p k m -> p n k m")
                    )
                wts[nb] = wt
                wts[nb + 1] = wt

            xrs = {}

            def load_x(rep, g):
                xt = xrpool.tile([P, xg, T_S], dt.bfloat16, tag="x",
                                 name=f"x{rep}_{g}")
                # x_on_act=True routes x via the ACT HWDGE ring to overlap
                # with the w stream on SP — measured NOT faster (A/B medians
                # 648 vs 698us), so default stays SP.
                eng = nc.scalar if x_on_act else nc.sync
                if chunk_first and rep == 0 and g == 0:
                    # tile 0 in per-ko slice-DMAs: with chunked w-pair-0 the
                    # first MM gates on ~0.75 MiB instead of 1.5 MiB (same
                    # proven slice-dep mechanism as the w chunking)
                    for c in range(xg):
                        eng.dma_start(xt[:, c : c + 1],
                                      xq[:, g * xg + c : g * xg + c + 1])
                else:
                    eng.dma_start(xt[:], xq[:, g * xg : (g + 1) * xg])
                xrs[g] = xt

            def xslice(ko, ct):
                return xrs[ko // xg][:, ko % xg, ct * TCH : (ct + 1) * TCH]

            bias_sb = cpool.tile([P, NB], dt.float32, tag="bias")

            for _rep in range(reps):
                # Prologue per rep: first w tiles and the resident x in PE
                # consumption order (x group g is first needed by ko=g*XG).
                load_w(_rep, 0)
                load_x(_rep, 0)
                load_x(_rep, 1)
                for g in range(2, n_xg):
                    load_x(_rep, g)
                if _rep == 0:
                    nc.sync.dma_start(bias_sb[:], bt)
                for nb in range(2, w_bufs, 2):
                    load_w(_rep, nb)

                for nb0 in range(0, NB, nb_par):
                    nbs = range(nb0, nb0 + nb_par)
                    for nb in nbs:
                        if nb % 2 == 0 and nb + w_bufs < NB:
                            load_w(_rep, nb + w_bufs)
                    pss = {
                        (nb, ct): ps.tile([P, TCH], dt.float32, tag="ps",
                                          name=f"ps{_rep}_{nb}_{ct}")
                        for nb in nbs
                        for ct in range(CT)
                    }
                    for ko in range(KO):
                        for nb in nbs:
                            for ct in range(CT):
                                nc.tensor.matmul(
                                    pss[nb, ct][:],
                                    wts[nb][:, nb % 2, ko],
                                    xslice(ko, ct),
                                    start=(ko == 0),
                                    stop=(ko == KO - 1),
                                )
                    # drain the whole nb-pair into one tile -> ONE out DMA
                    # (512 KiB) on the ACT HWDGE ring, decoupled from the
                    # x/w input stream on the SP ring.
                    o = opool.tile([P, nb_par, T_S], dt.bfloat16, tag="out",
                                   name=f"o{_rep}_{nb0}")
                    for i, nb in enumerate(nbs):
                        for ct in range(CT):
                            nc.scalar.add(o[:, i, ct * TCH : (ct + 1) * TCH],
                                          pss[nb, ct][:], bias_sb[:, nb : nb + 1])
                    nc.scalar.dma_start(
                        yt[nb0 : nb0 + nb_par].rearrange("n p t -> p n t"), o[:]
                    )
                    for nb in nbs:
                        del wts[nb]

    nc.compile()
    return nc


def _prep_inputs(x, weight, bias):
    """Host-side: threshold (fp32), mask+cast to bf16, DMA-friendly tiling."""
    flat_abs = np.abs(weight.reshape(-1))
    k = flat_abs.size - MAX_ITER
    thresh = float(np.partition(flat_abs, k)[k])

    w_eff = (weight * (np.abs(weight) >= thresh)).astype(BF16)

    # wq[nb, ki, ko, m] = w_eff[nb*128+m, ko*128+ki]  (identical on all cores)
    wq = np.ascontiguousarray(
        w_eff.reshape(NB, P, KO, P).transpose(0, 3, 2, 1)
    )
    # bt[p, nb] = bias[nb*128+p]
    bt = np.ascontiguousarray(bias.reshape(NB, P).T).astype(np.float32)

    xb = x.astype(BF16)
    in_maps = []
    for c in range(N_CORES):
        # xq[ki, ko, t] = x[c*1024 + t, ko*128 + ki]
        xs = xb[c * T_S : (c + 1) * T_S]  # [1024, 4096]
        xq = np.ascontiguousarray(xs.reshape(T_S, KO, P).transpose(2, 1, 0))
        in_maps.append({"xq": xq, "wq": wq, "bt": bt})
    return thresh, in_maps


def _unshard(results):
    # yt[nb, p, t] per core -> y[c*1024 + t, nb*128 + p]
    return np.ascontiguousarray(
        np.concatenate(
            [r["yt"].transpose(2, 0, 1).reshape(T_S, OUT_F).astype(np.float32)
             for r in results],
            axis=0,
        )
    )


def _run(x, weight, bias, **run_kwargs):
    x = np.asarray(x, dtype=np.float32)
    weight = np.asarray(weight, dtype=np.float32)
    bias = np.asarray(bias, dtype=np.float32)
    assert x.shape == (N_TOK, IN_F) and weight.shape == (OUT_F, IN_F)

    _, in_maps = _prep_inputs(x, weight, bias)
    nc = _build()
    res = run_bass_kernel_spmd(
        nc, in_maps, core_ids=list(range(N_CORES)), **run_kwargs
    )
    return _unshard(res.results), res


def kernel(x, weight, bias):
    y, _ = _run(x, weight, bias)
    return y
